# revision 8
# baseline (speedup 1.0000x reference)
"""Trainium2 Bass kernel for a 2-layer LSTM (64, 32) + MLP head.

Model (PyTorch semantics, eval mode):
    h1 = LSTM(4 -> 64)(x)            x: [B=4096, T=512, 4]
    h2 = LSTM(64 -> 32)(h1)
    y  = (relu(h2[:, -1] @ w_fc1.T + b_fc1)) @ w_fc2.T + b_fc2   # [B, 1]

Sharding: data-parallel over batch across 8 NeuronCores (512 rows each),
weights replicated.

v2 design (vs the v1 per-gate/PSUM-bank pipeline):
  * State kept transposed [units, batch]; layer-1 (64) and layer-2 (32)
    unit stacks fused to 96 rows with the 1-step layer-2 lag.
  * x folded INTO the recurrent matmul: the moving operand S is
    [101, F]: rows 0:96 h, row 96 ones (bias), rows 97:101 x_t.  One
    matmul per gate per step (K=101) instead of recurrent + x-proj
    pairs -- halves TensorE moving columns.
  * All 4 gates' matmuls write one contiguous PSUM tile [96, 4F], so
    ONE sigmoid activation instruction covers all four gates (ACT's
    ~185ns per-instruction access latency amortized 4x).  tanh(g) is
    computed as 2*sigmoid(2g)-1: the x2 is folded into g's weights, and
    the affine fix-up is a 4x-rate DVE tensor_scalar op.
  * Batch split into 2 independent chains of 256 (separate S/C/PSUM),
    interleaved in emission order so each engine works on one chain
    while the other chain's serial chain (mm -> sigmoid -> cell -> tanh
    -> h -> mm) is in flight on other engines.
  * S rotates over 4 buffers per chain; x_t DMA lands 4 steps ahead.
"""

import numpy as np
from contextlib import ExitStack

import concourse.bass as bass
import concourse.tile as tile
from concourse import bacc, mybir
from concourse import bass_utils
from concourse.alu_op_type import AluOpType

AF = mybir.ActivationFunctionType

B, T, D_IN, H1, H2 = 4096, 512, 4, 64, 32
NCORES = 8
BL = B // NCORES  # 512 batch rows per core

F32 = mybir.dt.float32
DT = mybir.dt.bfloat16

HS = H1 + H2          # 96 stacked units
KR = HS + 1 + D_IN    # 101 = h rows + ones row + x rows
NCH = 2               # batch chains per core
FC = BL // NCH        # 256 free columns per chain
NSB = 4               # S rotation depth
NDUMMY = 7            # PE-warming filler matmuls per step


def _build(n_steps: int = T):
    nc = bacc.Bacc("TRN2", target_bir_lowering=False, debug=False)

    xT = nc.dram_tensor("xT", [n_steps * 4, BL], DT, kind="ExternalInput")
    wg = nc.dram_tensor("wg", [KR, 4 * HS], DT, kind="ExternalInput")
    wf1 = nc.dram_tensor("wf1", [HS + 1, 16], DT, kind="ExternalInput")
    wf2 = nc.dram_tensor("wf2", [16, 1], DT, kind="ExternalInput")
    bf2 = nc.dram_tensor("bf2", [1, 1], F32, kind="ExternalInput")
    out = nc.dram_tensor("out", [1, BL], F32, kind="ExternalOutput")

    with tile.TileContext(nc) as tc, ExitStack() as ctx:
        const = ctx.enter_context(tc.tile_pool(name="const", bufs=1))
        sg_pool = ctx.enter_context(tc.tile_pool(name="sg", bufs=2))
        cell = ctx.enter_context(tc.tile_pool(name="cell", bufs=2))

        W = const.tile([KR, 4 * HS], DT, tag="W")
        nc.sync.dma_start(W[:], wg.ap())
        WF1 = const.tile([HS + 1, 16], DT, tag="WF1")
        nc.sync.dma_start(WF1[:], wf1.ap())
        WF2 = const.tile([16, 1], DT, tag="WF2")
        nc.sync.dma_start(WF2[:], wf2.ap())
        BF2 = const.tile([1, 1], F32, tag="BF2")
        nc.sync.dma_start(BF2[:], bf2.ap())

        # Per-chain state: S rotation ring and cell state C.
        S = [[None] * NSB for _ in range(NCH)]
        C = [None] * NCH
        for ch in range(NCH):
            for j in range(NSB):
                Sj = const.tile([KR, FC], DT, tag=f"S{ch}_{j}")
                nc.vector.memset(Sj[:], 0.0)
                nc.vector.memset(Sj[HS : HS + 1, :], 1.0)
                S[ch][j] = Sj
            Cch = const.tile([HS, FC], DT, tag=f"C{ch}")
            nc.vector.memset(Cch[:], 0.0)
            C[ch] = Cch

        # Preload x for steps 0..NSB-1 into the rings.
        for j in range(min(NSB, n_steps)):
            for ch in range(NCH):
                nc.sync.dma_start(
                    S[ch][j][HS + 1 : KR, :],
                    xT.ap()[4 * j : 4 * j + 4, ch * FC : (ch + 1) * FC],
                )

        # Gate order in W columns: i, f, g, o (each HS=96 wide).
        GI, GF, GG, GO = 0, 1, 2, 3

        with tc.tile_pool(name="psum", bufs=1, space="PSUM") as psum:
            P = [psum.tile([HS, 4 * FC], F32, tag=f"P{ch}", name=f"P{ch}") for ch in range(NCH)]
            # Scratch bank for PE-warming filler matmuls: the tensor engine
            # drops to its mid p-state (1.2 GHz) unless it stays busy ~3us;
            # filler matmuls during the per-step gate-recurrence gap keep it
            # at 2.4 GHz, halving the real matmuls on the critical loop.
            PDUM = psum.tile([HS, 4 * HS], F32, tag="PDUM", name="PDUM")

            for k in range(n_steps + 1):
                cur = [S[ch][k % NSB] for ch in range(NCH)]
                nxt = [S[ch][(k + 1) % NSB] for ch in range(NCH)]

                # 1) Recurrent+input matmuls: all 4 gates into one PSUM tile.
                for ch in range(NCH):
                    for g in range(4):
                        nc.tensor.matmul(
                            P[ch][:, g * FC : (g + 1) * FC],
                            W[:, g * HS : (g + 1) * HS],
                            cur[ch][0:KR, :],
                            start=True,
                            stop=True,
                        )
                # PE-warming fillers (see PDUM above); they execute during
                # the recurrence gap while PE waits for h(t).
                for _ in range(NDUMMY):
                    nc.tensor.matmul(
                        PDUM[:, :], W[:, 0:HS], W[0:KR, 0 : 4 * HS],
                        start=True, stop=True,
                    )
                # Prefetch x for step k+NSB into the buffer just freed.
                if k + NSB < n_steps:
                    for ch in range(NCH):
                        nc.sync.dma_start(
                            cur[ch][HS + 1 : KR, :],
                            xT.ap()[
                                4 * (k + NSB) : 4 * (k + NSB) + 4,
                                ch * FC : (ch + 1) * FC,
                            ],
                        )

                # 2) One sigmoid over all four gates per chain.
                SG = [None] * NCH
                for ch in range(NCH):
                    SGc = sg_pool.tile([HS, 4 * FC], DT, tag=f"SG{ch}")
                    nc.scalar.activation(SGc[:], P[ch][:, :], AF.Sigmoid)
                    SG[ch] = SGc

                # 3+4) Cell update + h per chain.  DVE queue order per chain:
                # [V, g', U, c', h] -- V first (needs only sigma+C), h right
                # after c' so it runs as soon as tanh(c) lands instead of
                # queueing behind the other chain's cell ops.
                for ch in range(NCH):
                    Vc = cell.tile([HS, FC], DT, tag=f"V{ch}", name=f"V{ch}")
                    nc.vector.tensor_mul(
                        Vc[:], SG[ch][:, GF * FC : (GF + 1) * FC], C[ch][:]
                    )
                    GNc = cell.tile([HS, FC], DT, tag=f"GN{ch}", name=f"GN{ch}")
                    # g = tanh(g_pre) = 2*sigmoid(2*g_pre) - 1 (x2 in weights)
                    nc.vector.tensor_scalar(
                        GNc[:], SG[ch][:, GG * FC : (GG + 1) * FC],
                        2.0, 1.0, AluOpType.mult, AluOpType.subtract,
                    )
                    Uc = cell.tile([HS, FC], DT, tag=f"U{ch}", name=f"U{ch}")
                    nc.vector.tensor_mul(
                        Uc[:], SG[ch][:, GI * FC : (GI + 1) * FC], GNc[:]
                    )
                    nc.vector.tensor_add(C[ch][:], Uc[:], Vc[:])
                    TCc = cell.tile([HS, FC], DT, tag=f"TC{ch}", name=f"TC{ch}")
                    nc.scalar.activation(TCc[:], C[ch][:], AF.Tanh)
                    nc.vector.tensor_mul(
                        nxt[ch][0:HS, :], SG[ch][:, GO * FC : (GO + 1) * FC],
                        TCc[:],
                    )
                if k == 0:
                    # Wipe garbage layer-2 state from the pipeline warmup.
                    for ch in range(NCH):
                        nc.vector.memset(nxt[ch][H1:HS, :], 0.0)
                        nc.vector.memset(C[ch][H1:HS, :], 0.0)

        # MLP head on h2 of the final state (rows 64:96; wf1 zero over h1).
        final = [S[ch][(n_steps + 1) % NSB] for ch in range(NCH)]
        with tc.tile_pool(name="psum_head", bufs=1, space="PSUM") as psh:
            for ch in range(NCH):
                PF = psh.tile([16, FC], F32, tag=f"PF{ch}")
                nc.tensor.matmul(
                    PF[:], WF1[:, :], final[ch][0 : HS + 1, :],
                    start=True, stop=True,
                )
                Z = cell.tile([16, FC], DT, tag=f"Z{ch}")
                nc.scalar.activation(Z[:], PF[:], AF.Relu)
                PO = psh.tile([1, FC], F32, tag=f"PO{ch}")
                nc.tensor.matmul(PO[:], WF2[:, :], Z[:], start=True, stop=True)
                Y = cell.tile([1, FC], F32, tag=f"Y{ch}")
                nc.scalar.activation(Y[:], PO[:], AF.Identity, bias=BF2[:, 0:1])
                nc.sync.dma_start(out.ap()[:, ch * FC : (ch + 1) * FC], Y[:])

    nc.compile()
    return nc


def _pack_weights(inputs, np_dt):
    w_ih1, w_hh1 = inputs["w_ih1"], inputs["w_hh1"]
    w_ih2, w_hh2 = inputs["w_ih2"], inputs["w_hh2"]
    b1 = (inputs["b_ih1"] + inputs["b_hh1"]).astype(np.float32)
    b2 = (inputs["b_ih2"] + inputs["b_hh2"]).astype(np.float32)
    # Fused gate weight blocks [KR=101, 4*HS].  Rows: h1(64), h2(32),
    # ones/bias(1), x(4).  Columns gate-major i,f,g,o; within a gate,
    # layer-1 units then layer-2 units.  Layer 1 uses h1+x rows; layer 2
    # uses h1 (=w_ih2) + h2 rows and no x.  g gate scaled by 2 so that
    # tanh(g) = 2*sigmoid(2g) - 1 needs only sigmoid.
    blocks = []
    for g in range(4):
        w1 = np.concatenate(
            [
                w_hh1[g * H1 : (g + 1) * H1, :].T,           # h1 rows [64,64]
                np.zeros((H2, H1), np.float32),              # h2 rows
                b1[None, g * H1 : (g + 1) * H1],             # bias row
                w_ih1[g * H1 : (g + 1) * H1, :].T,           # x rows [4,64]
            ],
            axis=0,
        )
        w2 = np.concatenate(
            [
                w_ih2[g * H2 : (g + 1) * H2, :].T,           # h1 rows [64,32]
                w_hh2[g * H2 : (g + 1) * H2, :].T,           # h2 rows [32,32]
                b2[None, g * H2 : (g + 1) * H2],             # bias row
                np.zeros((D_IN, H2), np.float32),            # x rows
            ],
            axis=0,
        )
        blk = np.concatenate([w1, w2], axis=1)               # [101, 96]
        if g == 2:
            blk = blk * 2.0
        blocks.append(blk)
    wg = np.concatenate(blocks, axis=1)                      # [101, 384]
    return {
        "wg": np.ascontiguousarray(wg).astype(np_dt),
        "wf1": np.ascontiguousarray(
            np.concatenate(
                [np.zeros((H1, 16), np.float32), inputs["w_fc1"].T,
                 inputs["b_fc1"][None, :]], axis=0)
        ).astype(np_dt),
        "wf2": np.ascontiguousarray(inputs["w_fc2"].T).astype(np_dt),
        "bf2": np.ascontiguousarray(inputs["b_fc2"][:, None]).astype(np.float32),
    }


_built = {}


def _get_nc(n_steps):
    if n_steps not in _built:
        _built[n_steps] = _build(n_steps)
    return _built[n_steps]


def _run(inputs, n_steps=T, **run_kwargs):
    np_dt = mybir.dt.np(DT)
    x = np.asarray(inputs["x"], np.float32)
    nb = x.shape[0]
    ncores = NCORES
    bl = nb // ncores
    assert bl == BL and x.shape[1] >= n_steps
    shared = _pack_weights(
        {k: np.asarray(v, np.float32) for k, v in inputs.items() if k != "x"},
        np_dt,
    )
    in_maps = []
    for c in range(ncores):
        xs = x[c * bl : (c + 1) * bl, :n_steps, :]  # [BL, T, 4]
        xT_np = np.ascontiguousarray(
            xs.transpose(1, 2, 0).reshape(n_steps * 4, bl)
        )
        in_maps.append(dict(shared, xT=xT_np.astype(np_dt)))
    nc = _get_nc(n_steps)
    res = bass_utils.run_bass_kernel_spmd(
        nc, in_maps, core_ids=list(range(ncores)), **run_kwargs
    )
    y = np.concatenate(
        [np.asarray(r["out"], np.float32).reshape(bl, 1) for r in res.results],
        axis=0,
    )
    return y, res


def kernel(**inputs) -> np.ndarray:
    y, _ = _run(inputs)
    return y


# revision 9
# speedup vs baseline: 1.0003x; 1.0003x over previous
"""Trainium2 Bass kernel for a 2-layer LSTM (64, 32) + MLP head.

Model (PyTorch semantics, eval mode):
    h1 = LSTM(4 -> 64)(x)            x: [B=4096, T=512, 4]
    h2 = LSTM(64 -> 32)(h1)
    y  = (relu(h2[:, -1] @ w_fc1.T + b_fc1)) @ w_fc2.T + b_fc2   # [B, 1]

Sharding: data-parallel over batch across 8 NeuronCores (512 rows each),
weights replicated.

v2 design (vs the v1 per-gate/PSUM-bank pipeline):
  * State kept transposed [units, batch]; layer-1 (64) and layer-2 (32)
    unit stacks fused to 96 rows with the 1-step layer-2 lag.
  * x folded INTO the recurrent matmul: the moving operand S is
    [101, F]: rows 0:96 h, row 96 ones (bias), rows 97:101 x_t.  One
    matmul per gate per step (K=101) instead of recurrent + x-proj
    pairs -- halves TensorE moving columns.
  * All 4 gates' matmuls write one contiguous PSUM tile [96, 4F], so
    ONE sigmoid activation instruction covers all four gates (ACT's
    ~185ns per-instruction access latency amortized 4x).  tanh(g) is
    computed as 2*sigmoid(2g)-1: the x2 is folded into g's weights, and
    the affine fix-up is a 4x-rate DVE tensor_scalar op.
  * Batch split into 2 independent chains of 256 (separate S/C/PSUM),
    interleaved in emission order so each engine works on one chain
    while the other chain's serial chain (mm -> sigmoid -> cell -> tanh
    -> h -> mm) is in flight on other engines.
  * S rotates over 4 buffers per chain; x_t DMA lands 4 steps ahead.
"""

import numpy as np
from contextlib import ExitStack

import concourse.bass as bass
import concourse.tile as tile
from concourse import bacc, mybir
from concourse import bass_utils
from concourse.alu_op_type import AluOpType

AF = mybir.ActivationFunctionType

B, T, D_IN, H1, H2 = 4096, 512, 4, 64, 32
NCORES = 8
BL = B // NCORES  # 512 batch rows per core

F32 = mybir.dt.float32
DT = mybir.dt.bfloat16

HS = H1 + H2          # 96 stacked units
KR = HS + 1 + D_IN    # 101 = h rows + ones row + x rows
NCH = 2               # batch chains per core
FC = BL // NCH        # 256 free columns per chain
NSB = 4               # S rotation depth
NDUMMY = 7            # PE-warming filler matmuls per step


def _build(n_steps: int = T):
    nc = bacc.Bacc("TRN2", target_bir_lowering=False, debug=False)

    xT = nc.dram_tensor("xT", [n_steps * 4, BL], DT, kind="ExternalInput")
    wg = nc.dram_tensor("wg", [KR, 4 * HS], DT, kind="ExternalInput")
    wf1 = nc.dram_tensor("wf1", [HS + 1, 16], DT, kind="ExternalInput")
    wf2 = nc.dram_tensor("wf2", [16, 1], DT, kind="ExternalInput")
    bf2 = nc.dram_tensor("bf2", [1, 1], F32, kind="ExternalInput")
    out = nc.dram_tensor("out", [1, BL], F32, kind="ExternalOutput")

    with tile.TileContext(nc) as tc, ExitStack() as ctx:
        const = ctx.enter_context(tc.tile_pool(name="const", bufs=1))
        sg_pool = ctx.enter_context(tc.tile_pool(name="sg", bufs=2))
        cell = ctx.enter_context(tc.tile_pool(name="cell", bufs=2))

        W = const.tile([KR, 4 * HS], DT, tag="W")
        nc.sync.dma_start(W[:], wg.ap())
        WF1 = const.tile([HS + 1, 16], DT, tag="WF1")
        nc.sync.dma_start(WF1[:], wf1.ap())
        WF2 = const.tile([16, 1], DT, tag="WF2")
        nc.sync.dma_start(WF2[:], wf2.ap())
        BF2 = const.tile([1, 1], F32, tag="BF2")
        nc.sync.dma_start(BF2[:], bf2.ap())

        # Per-chain state: S rotation ring and cell state C.
        S = [[None] * NSB for _ in range(NCH)]
        C = [None] * NCH
        for ch in range(NCH):
            for j in range(NSB):
                Sj = const.tile([KR, FC], DT, tag=f"S{ch}_{j}")
                nc.vector.memset(Sj[:], 0.0)
                nc.vector.memset(Sj[HS : HS + 1, :], 1.0)
                S[ch][j] = Sj
            Cch = const.tile([HS, FC], DT, tag=f"C{ch}")
            nc.vector.memset(Cch[:], 0.0)
            C[ch] = Cch

        # Preload x for steps 0..NSB-1 into the rings.
        for j in range(min(NSB, n_steps)):
            for ch in range(NCH):
                nc.sync.dma_start(
                    S[ch][j][HS + 1 : KR, :],
                    xT.ap()[4 * j : 4 * j + 4, ch * FC : (ch + 1) * FC],
                )

        # Gate order in W columns: i, f, g, o (each HS=96 wide).
        GI, GF, GG, GO = 0, 1, 2, 3

        with tc.tile_pool(name="psum", bufs=1, space="PSUM") as psum:
            P = [psum.tile([HS, 4 * FC], F32, tag=f"P{ch}", name=f"P{ch}") for ch in range(NCH)]
            # Scratch bank for PE-warming filler matmuls: the tensor engine
            # drops to its mid p-state (1.2 GHz) unless it stays busy ~3us;
            # filler matmuls during the per-step gate-recurrence gap keep it
            # at 2.4 GHz, halving the real matmuls on the critical loop.
            PDUM = psum.tile([HS, 4 * HS], F32, tag="PDUM", name="PDUM")

            for k in range(n_steps + 1):
                cur = [S[ch][k % NSB] for ch in range(NCH)]
                nxt = [S[ch][(k + 1) % NSB] for ch in range(NCH)]

                # 1) Recurrent+input matmuls: all 4 gates into one PSUM tile.
                for ch in range(NCH):
                    for g in range(4):
                        nc.tensor.matmul(
                            P[ch][:, g * FC : (g + 1) * FC],
                            W[:, g * HS : (g + 1) * HS],
                            cur[ch][0:KR, :],
                            start=True,
                            stop=True,
                        )
                # PE-warming fillers (see PDUM above); they execute during
                # the recurrence gap while PE waits for h(t).
                for _ in range(NDUMMY):
                    nc.tensor.matmul(
                        PDUM[:, :], W[:, 0:HS], W[0:KR, 0 : 4 * HS],
                        start=True, stop=True,
                    )
                # Prefetch x for step k+NSB into the buffer just freed.
                if k + NSB < n_steps:
                    for ch in range(NCH):
                        nc.sync.dma_start(
                            cur[ch][HS + 1 : KR, :],
                            xT.ap()[
                                4 * (k + NSB) : 4 * (k + NSB) + 4,
                                ch * FC : (ch + 1) * FC,
                            ],
                        )

                # 2) One sigmoid over all four gates per chain.
                SG = [None] * NCH
                for ch in range(NCH):
                    SGc = sg_pool.tile([HS, 4 * FC], DT, tag=f"SG{ch}")
                    nc.scalar.activation(SGc[:], P[ch][:, :], AF.Sigmoid)
                    SG[ch] = SGc

                # 3+4) Cell update + h per chain.  DVE queue order per chain:
                # [V, g', U, c', h] -- V first (needs only sigma+C), h right
                # after c' so it runs as soon as tanh(c) lands instead of
                # queueing behind the other chain's cell ops.
                TC = [None] * NCH
                for ch in range(NCH):
                    Vc = cell.tile([HS, FC], DT, tag=f"V{ch}", name=f"V{ch}")
                    nc.vector.tensor_mul(
                        Vc[:], SG[ch][:, GF * FC : (GF + 1) * FC], C[ch][:]
                    )
                    GNc = cell.tile([HS, FC], DT, tag=f"GN{ch}", name=f"GN{ch}")
                    # g = tanh(g_pre) = 2*sigmoid(2*g_pre) - 1 (x2 in weights)
                    nc.vector.tensor_scalar(
                        GNc[:], SG[ch][:, GG * FC : (GG + 1) * FC],
                        2.0, 1.0, AluOpType.mult, AluOpType.subtract,
                    )
                    Uc = cell.tile([HS, FC], DT, tag=f"U{ch}", name=f"U{ch}")
                    nc.vector.tensor_mul(
                        Uc[:], SG[ch][:, GI * FC : (GI + 1) * FC], GNc[:]
                    )
                    nc.vector.tensor_add(C[ch][:], Uc[:], Vc[:])
                    TCc = cell.tile([HS, FC], DT, tag=f"TC{ch}", name=f"TC{ch}")
                    nc.scalar.activation(TCc[:], C[ch][:], AF.Tanh)
                    TC[ch] = TCc
                for ch in range(NCH):
                    nc.vector.tensor_mul(
                        nxt[ch][0:HS, :], SG[ch][:, GO * FC : (GO + 1) * FC],
                        TC[ch][:],
                    )
                if k == 0:
                    # Wipe garbage layer-2 state from the pipeline warmup.
                    for ch in range(NCH):
                        nc.vector.memset(nxt[ch][H1:HS, :], 0.0)
                        nc.vector.memset(C[ch][H1:HS, :], 0.0)

        # MLP head on h2 of the final state (rows 64:96; wf1 zero over h1).
        final = [S[ch][(n_steps + 1) % NSB] for ch in range(NCH)]
        with tc.tile_pool(name="psum_head", bufs=1, space="PSUM") as psh:
            for ch in range(NCH):
                PF = psh.tile([16, FC], F32, tag=f"PF{ch}")
                nc.tensor.matmul(
                    PF[:], WF1[:, :], final[ch][0 : HS + 1, :],
                    start=True, stop=True,
                )
                Z = cell.tile([16, FC], DT, tag=f"Z{ch}")
                nc.scalar.activation(Z[:], PF[:], AF.Relu)
                PO = psh.tile([1, FC], F32, tag=f"PO{ch}")
                nc.tensor.matmul(PO[:], WF2[:, :], Z[:], start=True, stop=True)
                Y = cell.tile([1, FC], F32, tag=f"Y{ch}")
                nc.scalar.activation(Y[:], PO[:], AF.Identity, bias=BF2[:, 0:1])
                nc.sync.dma_start(out.ap()[:, ch * FC : (ch + 1) * FC], Y[:])

    nc.compile()
    return nc


def _pack_weights(inputs, np_dt):
    w_ih1, w_hh1 = inputs["w_ih1"], inputs["w_hh1"]
    w_ih2, w_hh2 = inputs["w_ih2"], inputs["w_hh2"]
    b1 = (inputs["b_ih1"] + inputs["b_hh1"]).astype(np.float32)
    b2 = (inputs["b_ih2"] + inputs["b_hh2"]).astype(np.float32)
    # Fused gate weight blocks [KR=101, 4*HS].  Rows: h1(64), h2(32),
    # ones/bias(1), x(4).  Columns gate-major i,f,g,o; within a gate,
    # layer-1 units then layer-2 units.  Layer 1 uses h1+x rows; layer 2
    # uses h1 (=w_ih2) + h2 rows and no x.  g gate scaled by 2 so that
    # tanh(g) = 2*sigmoid(2g) - 1 needs only sigmoid.
    blocks = []
    for g in range(4):
        w1 = np.concatenate(
            [
                w_hh1[g * H1 : (g + 1) * H1, :].T,           # h1 rows [64,64]
                np.zeros((H2, H1), np.float32),              # h2 rows
                b1[None, g * H1 : (g + 1) * H1],             # bias row
                w_ih1[g * H1 : (g + 1) * H1, :].T,           # x rows [4,64]
            ],
            axis=0,
        )
        w2 = np.concatenate(
            [
                w_ih2[g * H2 : (g + 1) * H2, :].T,           # h1 rows [64,32]
                w_hh2[g * H2 : (g + 1) * H2, :].T,           # h2 rows [32,32]
                b2[None, g * H2 : (g + 1) * H2],             # bias row
                np.zeros((D_IN, H2), np.float32),            # x rows
            ],
            axis=0,
        )
        blk = np.concatenate([w1, w2], axis=1)               # [101, 96]
        if g == 2:
            blk = blk * 2.0
        blocks.append(blk)
    wg = np.concatenate(blocks, axis=1)                      # [101, 384]
    return {
        "wg": np.ascontiguousarray(wg).astype(np_dt),
        "wf1": np.ascontiguousarray(
            np.concatenate(
                [np.zeros((H1, 16), np.float32), inputs["w_fc1"].T,
                 inputs["b_fc1"][None, :]], axis=0)
        ).astype(np_dt),
        "wf2": np.ascontiguousarray(inputs["w_fc2"].T).astype(np_dt),
        "bf2": np.ascontiguousarray(inputs["b_fc2"][:, None]).astype(np.float32),
    }


_built = {}


def _get_nc(n_steps):
    if n_steps not in _built:
        _built[n_steps] = _build(n_steps)
    return _built[n_steps]


def _run(inputs, n_steps=T, **run_kwargs):
    np_dt = mybir.dt.np(DT)
    x = np.asarray(inputs["x"], np.float32)
    nb = x.shape[0]
    ncores = NCORES
    bl = nb // ncores
    assert bl == BL and x.shape[1] >= n_steps
    shared = _pack_weights(
        {k: np.asarray(v, np.float32) for k, v in inputs.items() if k != "x"},
        np_dt,
    )
    in_maps = []
    for c in range(ncores):
        xs = x[c * bl : (c + 1) * bl, :n_steps, :]  # [BL, T, 4]
        xT_np = np.ascontiguousarray(
            xs.transpose(1, 2, 0).reshape(n_steps * 4, bl)
        )
        in_maps.append(dict(shared, xT=xT_np.astype(np_dt)))
    nc = _get_nc(n_steps)
    res = bass_utils.run_bass_kernel_spmd(
        nc, in_maps, core_ids=list(range(ncores)), **run_kwargs
    )
    y = np.concatenate(
        [np.asarray(r["out"], np.float32).reshape(bl, 1) for r in res.results],
        axis=0,
    )
    return y, res


def kernel(**inputs) -> np.ndarray:
    y, _ = _run(inputs)
    return y


# revision 10
# speedup vs baseline: 1.0301x; 1.0298x over previous
"""Trainium2 Bass kernel for a 2-layer LSTM (64, 32) + MLP head.

Model (PyTorch semantics, eval mode):
    h1 = LSTM(4 -> 64)(x)            x: [B=4096, T=512, 4]
    h2 = LSTM(64 -> 32)(h1)
    y  = (relu(h2[:, -1] @ w_fc1.T + b_fc1)) @ w_fc2.T + b_fc2   # [B, 1]

Sharding: data-parallel over batch across 8 NeuronCores (512 rows each),
weights replicated.

v2 design (vs the v1 per-gate/PSUM-bank pipeline):
  * State kept transposed [units, batch]; layer-1 (64) and layer-2 (32)
    unit stacks fused to 96 rows with the 1-step layer-2 lag.
  * x folded INTO the recurrent matmul: the moving operand S is
    [101, F]: rows 0:96 h, row 96 ones (bias), rows 97:101 x_t.  One
    matmul per gate per step (K=101) instead of recurrent + x-proj
    pairs -- halves TensorE moving columns.
  * All 4 gates' matmuls write one contiguous PSUM tile [96, 4F], so
    ONE sigmoid activation instruction covers all four gates (ACT's
    ~185ns per-instruction access latency amortized 4x).  tanh(g) is
    computed as 2*sigmoid(2g)-1: the x2 is folded into g's weights, and
    the affine fix-up is a 4x-rate DVE tensor_scalar op.
  * Batch split into 2 independent chains of 256 (separate S/C/PSUM),
    interleaved in emission order so each engine works on one chain
    while the other chain's serial chain (mm -> sigmoid -> cell -> tanh
    -> h -> mm) is in flight on other engines.
  * S rotates over 4 buffers per chain; x_t DMA lands 4 steps ahead.
"""

import numpy as np
from contextlib import ExitStack

import concourse.bass as bass
import concourse.tile as tile
from concourse import bacc, mybir
from concourse import bass_utils
from concourse.alu_op_type import AluOpType

AF = mybir.ActivationFunctionType

B, T, D_IN, H1, H2 = 4096, 512, 4, 64, 32
NCORES = 8
BL = B // NCORES  # 512 batch rows per core

F32 = mybir.dt.float32
DT = mybir.dt.bfloat16

HS = H1 + H2          # 96 stacked units
KR = HS + 1 + D_IN    # 101 = h rows + ones row + x rows
NCH = 2               # batch chains per core
FC = BL // NCH        # 256 free columns per chain
NSB = 4               # S rotation depth
NDUMMY = 7            # PE-warming filler matmuls per step


def _build(n_steps: int = T):
    nc = bacc.Bacc("TRN2", target_bir_lowering=False, debug=False)

    xT = nc.dram_tensor("xT", [n_steps * 4, BL], DT, kind="ExternalInput")
    wg = nc.dram_tensor("wg", [KR, 4 * HS], DT, kind="ExternalInput")
    wf1 = nc.dram_tensor("wf1", [HS + 1, 16], DT, kind="ExternalInput")
    wf2 = nc.dram_tensor("wf2", [16, 1], DT, kind="ExternalInput")
    bf2 = nc.dram_tensor("bf2", [1, 1], F32, kind="ExternalInput")
    out = nc.dram_tensor("out", [1, BL], F32, kind="ExternalOutput")

    with tile.TileContext(nc) as tc, ExitStack() as ctx:
        const = ctx.enter_context(tc.tile_pool(name="const", bufs=1))
        sg_pool = ctx.enter_context(tc.tile_pool(name="sg", bufs=2))
        cell = ctx.enter_context(tc.tile_pool(name="cell", bufs=2))

        W = const.tile([KR, 4 * HS], DT, tag="W")
        nc.sync.dma_start(W[:], wg.ap())
        WF1 = const.tile([HS + 1, 16], DT, tag="WF1")
        nc.sync.dma_start(WF1[:], wf1.ap())
        WF2 = const.tile([16, 1], DT, tag="WF2")
        nc.sync.dma_start(WF2[:], wf2.ap())
        BF2 = const.tile([1, 1], F32, tag="BF2")
        nc.sync.dma_start(BF2[:], bf2.ap())

        # Per-chain state: S rotation ring and cell state C.
        S = [[None] * NSB for _ in range(NCH)]
        C = [None] * NCH
        for ch in range(NCH):
            for j in range(NSB):
                Sj = const.tile([KR, FC], DT, tag=f"S{ch}_{j}")
                nc.vector.memset(Sj[:], 0.0)
                nc.vector.memset(Sj[HS : HS + 1, :], 1.0)
                S[ch][j] = Sj
            Cch = const.tile([HS, FC], DT, tag=f"C{ch}")
            nc.vector.memset(Cch[:], 0.0)
            C[ch] = Cch

        # Preload x for steps 0..NSB-1 into the rings.
        for j in range(min(NSB, n_steps)):
            for ch in range(NCH):
                nc.sync.dma_start(
                    S[ch][j][HS + 1 : KR, :],
                    xT.ap()[4 * j : 4 * j + 4, ch * FC : (ch + 1) * FC],
                )

        # Gate order in W columns: i, f, g, o (each HS=96 wide).
        GI, GF, GG, GO = 0, 1, 2, 3

        with tc.tile_pool(name="psum", bufs=1, space="PSUM") as psum:
            P = [psum.tile([HS, 4 * FC], F32, tag=f"P{ch}", name=f"P{ch}") for ch in range(NCH)]
            # Scratch bank for PE-warming filler matmuls: the tensor engine
            # drops to its mid p-state (1.2 GHz) unless it stays busy ~3us;
            # filler matmuls during the per-step gate-recurrence gap keep it
            # at 2.4 GHz, halving the real matmuls on the critical loop.
            PDUM = psum.tile([HS, 4 * HS], F32, tag="PDUM", name="PDUM")

            for k in range(n_steps + 1):
                cur = [S[ch][k % NSB] for ch in range(NCH)]
                nxt = [S[ch][(k + 1) % NSB] for ch in range(NCH)]

                # 1) Recurrent+input matmuls: all 4 gates into one PSUM tile.
                for ch in range(NCH):
                    for g in range(4):
                        nc.tensor.matmul(
                            P[ch][:, g * FC : (g + 1) * FC],
                            W[:, g * HS : (g + 1) * HS],
                            cur[ch][0:KR, :],
                            start=True,
                            stop=True,
                        )
                # PE-warming fillers (see PDUM above); they execute during
                # the recurrence gap while PE waits for h(t).
                for _ in range(NDUMMY):
                    nc.tensor.matmul(
                        PDUM[:, :], W[:, 0:HS], W[0:KR, 0 : 4 * HS],
                        start=True, stop=True,
                    )
                # Prefetch x for step k+NSB into the buffer just freed.
                if k + NSB < n_steps:
                    for ch in range(NCH):
                        nc.sync.dma_start(
                            cur[ch][HS + 1 : KR, :],
                            xT.ap()[
                                4 * (k + NSB) : 4 * (k + NSB) + 4,
                                ch * FC : (ch + 1) * FC,
                            ],
                        )

                # 2) One sigmoid over all four gates per chain.
                SG = [None] * NCH
                for ch in range(NCH):
                    SGc = sg_pool.tile([HS, 4 * FC], DT, tag=f"SG{ch}")
                    nc.scalar.activation(SGc[:], P[ch][:, :], AF.Sigmoid)
                    SG[ch] = SGc

                # 3+4) Cell update + h per chain.  DVE queue order per chain:
                # [V, g', U, c', h] -- V first (needs only sigma+C), h right
                # after c' so it runs as soon as tanh(c) lands instead of
                # queueing behind the other chain's cell ops.
                GN = [None] * NCH
                for ch in range(NCH):
                    GNc = cell.tile([HS, FC], DT, tag=f"GN{ch}", name=f"GN{ch}")
                    # g = tanh(g_pre) = 2*sigmoid(2*g_pre) - 1 (x2 in weights)
                    nc.vector.tensor_scalar(
                        GNc[:], SG[ch][:, GG * FC : (GG + 1) * FC],
                        2.0, 1.0, AluOpType.mult, AluOpType.subtract,
                    )
                    GN[ch] = GNc
                for ch in range(NCH):
                    Uc = cell.tile([HS, FC], DT, tag=f"U{ch}", name=f"U{ch}")
                    nc.vector.tensor_mul(
                        Uc[:], SG[ch][:, GI * FC : (GI + 1) * FC], GN[ch][:]
                    )
                    Vc = cell.tile([HS, FC], DT, tag=f"V{ch}", name=f"V{ch}")
                    nc.vector.tensor_mul(
                        Vc[:], SG[ch][:, GF * FC : (GF + 1) * FC], C[ch][:]
                    )
                    nc.vector.tensor_add(C[ch][:], Uc[:], Vc[:])

                TC = [None] * NCH
                for ch in range(NCH):
                    TCc = cell.tile([HS, FC], DT, tag=f"TC{ch}", name=f"TC{ch}")
                    nc.scalar.activation(TCc[:], C[ch][:], AF.Tanh)
                    TC[ch] = TCc
                for ch in range(NCH):
                    nc.vector.tensor_mul(
                        nxt[ch][0:HS, :], SG[ch][:, GO * FC : (GO + 1) * FC],
                        TC[ch][:],
                    )
                if k == 0:
                    # Wipe garbage layer-2 state from the pipeline warmup.
                    for ch in range(NCH):
                        nc.vector.memset(nxt[ch][H1:HS, :], 0.0)
                        nc.vector.memset(C[ch][H1:HS, :], 0.0)

        # MLP head on h2 of the final state (rows 64:96; wf1 zero over h1).
        final = [S[ch][(n_steps + 1) % NSB] for ch in range(NCH)]
        with tc.tile_pool(name="psum_head", bufs=1, space="PSUM") as psh:
            for ch in range(NCH):
                PF = psh.tile([16, FC], F32, tag=f"PF{ch}")
                nc.tensor.matmul(
                    PF[:], WF1[:, :], final[ch][0 : HS + 1, :],
                    start=True, stop=True,
                )
                Z = cell.tile([16, FC], DT, tag=f"Z{ch}")
                nc.scalar.activation(Z[:], PF[:], AF.Relu)
                PO = psh.tile([1, FC], F32, tag=f"PO{ch}")
                nc.tensor.matmul(PO[:], WF2[:, :], Z[:], start=True, stop=True)
                Y = cell.tile([1, FC], F32, tag=f"Y{ch}")
                nc.scalar.activation(Y[:], PO[:], AF.Identity, bias=BF2[:, 0:1])
                nc.sync.dma_start(out.ap()[:, ch * FC : (ch + 1) * FC], Y[:])

    nc.compile()
    return nc


def _pack_weights(inputs, np_dt):
    w_ih1, w_hh1 = inputs["w_ih1"], inputs["w_hh1"]
    w_ih2, w_hh2 = inputs["w_ih2"], inputs["w_hh2"]
    b1 = (inputs["b_ih1"] + inputs["b_hh1"]).astype(np.float32)
    b2 = (inputs["b_ih2"] + inputs["b_hh2"]).astype(np.float32)
    # Fused gate weight blocks [KR=101, 4*HS].  Rows: h1(64), h2(32),
    # ones/bias(1), x(4).  Columns gate-major i,f,g,o; within a gate,
    # layer-1 units then layer-2 units.  Layer 1 uses h1+x rows; layer 2
    # uses h1 (=w_ih2) + h2 rows and no x.  g gate scaled by 2 so that
    # tanh(g) = 2*sigmoid(2g) - 1 needs only sigmoid.
    blocks = []
    for g in range(4):
        w1 = np.concatenate(
            [
                w_hh1[g * H1 : (g + 1) * H1, :].T,           # h1 rows [64,64]
                np.zeros((H2, H1), np.float32),              # h2 rows
                b1[None, g * H1 : (g + 1) * H1],             # bias row
                w_ih1[g * H1 : (g + 1) * H1, :].T,           # x rows [4,64]
            ],
            axis=0,
        )
        w2 = np.concatenate(
            [
                w_ih2[g * H2 : (g + 1) * H2, :].T,           # h1 rows [64,32]
                w_hh2[g * H2 : (g + 1) * H2, :].T,           # h2 rows [32,32]
                b2[None, g * H2 : (g + 1) * H2],             # bias row
                np.zeros((D_IN, H2), np.float32),            # x rows
            ],
            axis=0,
        )
        blk = np.concatenate([w1, w2], axis=1)               # [101, 96]
        if g == 2:
            blk = blk * 2.0
        blocks.append(blk)
    wg = np.concatenate(blocks, axis=1)                      # [101, 384]
    return {
        "wg": np.ascontiguousarray(wg).astype(np_dt),
        "wf1": np.ascontiguousarray(
            np.concatenate(
                [np.zeros((H1, 16), np.float32), inputs["w_fc1"].T,
                 inputs["b_fc1"][None, :]], axis=0)
        ).astype(np_dt),
        "wf2": np.ascontiguousarray(inputs["w_fc2"].T).astype(np_dt),
        "bf2": np.ascontiguousarray(inputs["b_fc2"][:, None]).astype(np.float32),
    }


_built = {}


def _get_nc(n_steps):
    if n_steps not in _built:
        _built[n_steps] = _build(n_steps)
    return _built[n_steps]


def _run(inputs, n_steps=T, **run_kwargs):
    np_dt = mybir.dt.np(DT)
    x = np.asarray(inputs["x"], np.float32)
    nb = x.shape[0]
    ncores = NCORES
    bl = nb // ncores
    assert bl == BL and x.shape[1] >= n_steps
    shared = _pack_weights(
        {k: np.asarray(v, np.float32) for k, v in inputs.items() if k != "x"},
        np_dt,
    )
    in_maps = []
    for c in range(ncores):
        xs = x[c * bl : (c + 1) * bl, :n_steps, :]  # [BL, T, 4]
        xT_np = np.ascontiguousarray(
            xs.transpose(1, 2, 0).reshape(n_steps * 4, bl)
        )
        in_maps.append(dict(shared, xT=xT_np.astype(np_dt)))
    nc = _get_nc(n_steps)
    res = bass_utils.run_bass_kernel_spmd(
        nc, in_maps, core_ids=list(range(ncores)), **run_kwargs
    )
    y = np.concatenate(
        [np.asarray(r["out"], np.float32).reshape(bl, 1) for r in res.results],
        axis=0,
    )
    return y, res


def kernel(**inputs) -> np.ndarray:
    y, _ = _run(inputs)
    return y
